# revision 16
# baseline (speedup 1.0000x reference)
"""Distributed Trainium2 (Bass/Tile) kernel for nn_Anchor_Loss2.

Math: the reference computes
    dist[i,j] = (||x_i||^2 - 2 x_i.a_j + ||a_j||^2) / D
    S = segment_sum(dist, y); M = S / max(cnt,1)
    loss = sum_{l present} (2 M[l,l] - sum_j M[l,j])

Expanding the segment sum, only per-class aggregates are needed:
    cnt_l, sx2_l = sum_{i in l} ||x_i||^2, SX_l = sum_{i in l} x_i
    S[l,j]*D   = sx2_l - 2 SX_l.a_j + cnt_l*||a_j||^2
    rowsum_l*D = C*sx2_l - 2 SX_l.asum + cnt_l*a2sum
so the big GEMM x@anchors.T disappears; the dominant work is the
segment-sum of x (done as a one-hot matmul on TensorE).

Sharding: rows are assigned to cores BY CLASS (each core owns a
contiguous window of <=127 classes), so all rows of a class land on one
core and every per-class aggregate is fully local. Cross-core comm is
only (a) an all-reduce of the anchor column-sums (asum / a2sum come from
the full anchor set) and (b) an all-reduce of the final scalar loss.
Row->core assignment is a free choice of sharding since the loss is
permutation invariant in the rows.
"""

import functools
import os
import sys

import numpy as np

for _p in ("/opt/trn_rl_repo",):
    if _p not in sys.path:
        sys.path.insert(0, _p)

N_CORES = 8
C = 1000
D = 1024
PAD_SLOT = 127  # local class slot used for padding rows (always masked)
CHUNKS_PER_DMA = 4

LAST_EXEC_NS = None
LAST_RESULTS = None


@functools.lru_cache(maxsize=8)
def _build(nchunks: int, stage: int = 99):
    import concourse.bass as bass  # noqa: F401
    import concourse.mybir as mybir
    import concourse.tile as tile
    from concourse import bacc

    dt = mybir.dt
    f32 = dt.float32
    bf16 = dt.bfloat16
    i32 = dt.int32
    Alu = mybir.AluOpType
    AX = mybir.AxisListType

    R = nchunks * 128
    G = CHUNKS_PER_DMA
    assert nchunks % G == 0
    ngroups = nchunks // G

    nc = bacc.Bacc("TRN2", target_bir_lowering=False, debug=False,
                   num_devices=N_CORES)

    x_d = nc.dram_tensor("x", [R, D], f32, kind="ExternalInput")
    y_d = nc.dram_tensor("y", [128, nchunks], f32, kind="ExternalInput")
    al_d = nc.dram_tensor("al", [128, D], f32, kind="ExternalInput")
    out_d = nc.dram_tensor("out", [1, 1], f32, kind="ExternalOutput")

    RG = [list(range(N_CORES))]

    def _graph(tc):
        with (
            tc.tile_pool(name="const", bufs=1) as constp,
            tc.tile_pool(name="anch", bufs=1) as anchp,
            tc.tile_pool(name="xin", bufs=3) as xinp,
            tc.tile_pool(name="xbf", bufs=3) as xbfp,
            tc.tile_pool(name="sq", bufs=2) as sqp,
            tc.tile_pool(name="oh", bufs=4) as ohp,
            tc.tile_pool(name="sm", bufs=6) as smp,
            tc.tile_pool(name="ep", bufs=1) as epp,
            tc.tile_pool(name="psA", bufs=1, space="PSUM") as psA,
            tc.tile_pool(name="psB", bufs=1, space="PSUM") as psB,
            tc.tile_pool(name="dram", bufs=1, space="DRAM") as dramp,
        ):
            # ---------------- constants ----------------
            iota_i = constp.tile([128, 128], i32, name="iota_i")
            nc.gpsimd.iota(iota_i[:], pattern=[[1, 128]], base=0,
                           channel_multiplier=0)
            iota_f = constp.tile([128, 128], f32, name="iota_f")
            nc.vector.tensor_copy(iota_f[:], iota_i[:])
            ones_bf = constp.tile([128, 1], bf16, name="ones_bf")
            nc.vector.memset(ones_bf[:], 1.0)
            ones_f = constp.tile([128, 1], f32, name="ones_f")
            nc.vector.memset(ones_f[:], 1.0)
            pidx_i = constp.tile([128, 1], i32, name="pidx_i")
            nc.gpsimd.iota(pidx_i[:], pattern=[[1, 1]], base=0,
                           channel_multiplier=1)
            pidx_f = constp.tile([128, 1], f32, name="pidx_f")
            nc.vector.tensor_copy(pidx_f[:], pidx_i[:])
            padmask = constp.tile([128, 1], f32, name="padmask")
            nc.vector.tensor_scalar(padmask[:], pidx_f[:],
                                    float(PAD_SLOT) - 0.5, None,
                                    op0=Alu.is_le)
            y_sb = constp.tile([128, nchunks], f32, name="y_sb")
            nc.sync.dma_start(y_sb[:], y_d[:])

            # ---------- anchors: local slice + colsum all-reduce ----------
            A = anchp.tile([128, D], f32, name="A")
            nc.sync.dma_start(A[:], al_d[:])
            A_bf = anchp.tile([128, D], bf16, name="A_bf")
            nc.scalar.copy(A_bf[:], A[:])
            Asq_bf = anchp.tile([128, D], bf16, name="Asq_bf")
            nc.vector.tensor_tensor(Asq_bf[:], A_bf[:], A_bf[:], op=Alu.mult)
            cs_sb = anchp.tile([1, 2 * D], f32, name="cs_sb")
            for h in range(2):
                pcsa = psB.tile([1, 512], f32, tag="pcs", bufs=2,
                                name=f"pcsa{h}")
                nc.tensor.matmul(pcsa[:], ones_bf[:],
                                 A_bf[:, h * 512:(h + 1) * 512])
                nc.vector.tensor_copy(cs_sb[:, h * 512:(h + 1) * 512], pcsa[:])
            for h in range(2):
                pcsq = psB.tile([1, 512], f32, tag="pcs", bufs=2,
                                name=f"pcsq{h}")
                nc.tensor.matmul(pcsq[:], ones_bf[:],
                                 Asq_bf[:, h * 512:(h + 1) * 512])
                nc.vector.tensor_copy(cs_sb[:, D + h * 512:D + (h + 1) * 512],
                                      pcsq[:])
            if stage < 2:
                nc.sync.dma_start(out_d[:], cs_sb[:, 0:1])
                return
            cc_in = dramp.tile([1, 2 * D], f32, name="cc_in")
            cc_out = dramp.tile([1, 2 * D], f32, name="cc_out",
                                addr_space="Shared")
            nc.gpsimd.dma_start(cc_in[:], cs_sb[:])
            nc.gpsimd.collective_compute(
                "AllReduce", Alu.add, replica_groups=RG,
                ins=[cc_in.opt()], outs=[cc_out.opt()])
            ar_sb = anchp.tile([1, 2 * D], f32, name="ar_sb")
            nc.gpsimd.dma_start(ar_sb[:], cc_out[:])
            asum_bc = anchp.tile([128, D], f32, name="asum_bc")
            nc.gpsimd.partition_broadcast(asum_bc[:], ar_sb[:, 0:D])
            a2sum = anchp.tile([1, 1], f32, name="a2sum")
            nc.vector.tensor_reduce(a2sum[:], ar_sb[:, D:2 * D], axis=AX.X,
                                    op=Alu.add)

            if stage < 3:
                nc.sync.dma_start(out_d[:], a2sum[:])
                return
            # ---------------- main streaming accumulation ----------------
            p_sx0 = psA.tile([128, 512], f32, tag="sx0", name="p_sx0")
            p_sx1 = psA.tile([128, 512], f32, tag="sx1", name="p_sx1")
            p_sc = psA.tile([128, 2], f32, tag="sc", name="p_sc")

            x_r = x_d.ap().rearrange("(g t p) d -> g p t d", t=G, p=128)
            last_xb = [None]
            for g in range(ngroups):
                xt = xinp.tile([128, G, D], f32, name="xt")
                nc.sync.dma_start(xt[:], x_r[g])
                xb = xbfp.tile([128, G, D], bf16, name="xb")
                nc.scalar.copy(xb[:], xt[:])
                last_xb[0] = xb
                for t in range(G):
                    k = g * G + t
                    st, sp = (k == 0), (k == nchunks - 1)
                    if stage >= 32:
                        oh_t = ohp.tile([128, 128], bf16, name="oh_t")
                        nc.vector.tensor_scalar(oh_t[:], iota_f[:],
                                                y_sb[:, k:k + 1], None,
                                                op0=Alu.is_equal)
                    if stage >= 33:
                        xsq = sqp.tile([128, D], bf16, name="xsq")
                        x2c = smp.tile([128, 1], f32, name="x2c")
                        nc.scalar.activation(
                            xsq[:], xb[:, t, :],
                            mybir.ActivationFunctionType.Square,
                            accum_out=x2c[:])
                        rhs2 = smp.tile([128, 2], bf16, name="rhs2")
                        # center: store (x2 - D) so the bf16 cast only sees
                        # the fluctuation; sx2 is rebuilt as D*cnt + sum
                        nc.vector.tensor_scalar_add(rhs2[:, 0:1], x2c[:],
                                                    -float(D))
                        nc.vector.memset(rhs2[:, 1:2], 1.0)
                    if stage >= 34:
                        nc.tensor.matmul(p_sx0[:], oh_t[:], xb[:, t, 0:512],
                                         start=st, stop=sp)
                        nc.tensor.matmul(p_sx1[:], oh_t[:], xb[:, t, 512:1024],
                                         start=st, stop=sp)
                        nc.tensor.matmul(p_sc[:], oh_t[:], rhs2[:],
                                         start=st, stop=sp)
            if stage < 34:
                res31 = epp.tile([1, 1], f32, name="res31")
                nc.vector.tensor_copy(res31[:], last_xb[0][0:1, 0, 0:1])
                nc.sync.dma_start(out_d[:], res31[:])
                return

            # ---------------- epilogue (per-class -> scalar) ----------------
            SX = epp.tile([128, D], f32, name="SX")
            nc.vector.tensor_copy(SX[:, 0:512], p_sx0[:])
            nc.vector.tensor_copy(SX[:, 512:1024], p_sx1[:])
            if stage < 4:
                nc.sync.dma_start(out_d[:], SX[0:1, 0:1])
                return
            x2cs = epp.tile([128, 1], f32, name="x2cs")
            nc.vector.tensor_copy(x2cs[:], p_sc[:, 0:1])
            cnt = epp.tile([128, 1], f32, name="cnt")
            nc.vector.tensor_copy(cnt[:], p_sc[:, 1:2])
            # sx2 = D*cnt + sum((x2 - D)) : undo the centering
            sx2 = epp.tile([128, 1], f32, name="sx2")
            nc.vector.scalar_tensor_tensor(sx2[:], cnt[:], float(D), x2cs[:],
                                           op0=Alu.mult, op1=Alu.add)
            scr = epp.tile([128, D], f32, name="scr")
            dotdiag = epp.tile([128, 1], f32, name="dotdiag")
            nc.vector.scalar_tensor_tensor(
                scr[:], SX[:], 1.0, A[:], op0=Alu.mult, op1=Alu.mult,
                accum_out=dotdiag[:])
            scr2 = epp.tile([128, D], f32, name="scr2")
            dotsum = epp.tile([128, 1], f32, name="dotsum")
            nc.vector.scalar_tensor_tensor(
                scr2[:], SX[:], 1.0, asum_bc[:], op0=Alu.mult, op1=Alu.mult,
                accum_out=dotsum[:])
            scr3 = epp.tile([128, D], f32, name="scr3")
            a2l = epp.tile([128, 1], f32, name="a2l")
            nc.vector.scalar_tensor_tensor(
                scr3[:], A[:], 1.0, A[:], op0=Alu.mult, op1=Alu.mult,
                accum_out=a2l[:])

            cntm = epp.tile([128, 1], f32, name="cntm")
            nc.vector.tensor_scalar_max(cntm[:], cnt[:], 1.0)
            inv = epp.tile([128, 1], f32, name="inv")
            nc.vector.reciprocal(inv[:], cntm[:])
            # t1 = sx2 - 2*dotdiag ; sll = t1 + cnt*a2l  (= S_ll * D)
            t1 = epp.tile([128, 1], f32, name="t1")
            nc.vector.scalar_tensor_tensor(t1[:], dotdiag[:], -2.0, sx2[:],
                                           op0=Alu.mult, op1=Alu.add)
            t2 = epp.tile([128, 1], f32, name="t2")
            nc.vector.tensor_tensor(t2[:], cnt[:], a2l[:], op=Alu.mult)
            sll = epp.tile([128, 1], f32, name="sll")
            nc.vector.tensor_tensor(sll[:], t1[:], t2[:], op=Alu.add)
            # rs = C*sx2 - 2*dotsum  (= rowsum*D minus the cnt*a2sum part)
            ds2 = epp.tile([128, 1], f32, name="ds2")
            nc.vector.tensor_scalar_mul(ds2[:], dotsum[:], 2.0)
            rs = epp.tile([128, 1], f32, name="rs")
            nc.vector.scalar_tensor_tensor(rs[:], sx2[:], float(C), ds2[:],
                                           op0=Alu.mult, op1=Alu.subtract)
            num = epp.tile([128, 1], f32, name="num")
            nc.vector.scalar_tensor_tensor(num[:], sll[:], 2.0, rs[:],
                                           op0=Alu.mult, op1=Alu.subtract)
            pl = epp.tile([128, 1], f32, name="pl")
            nc.vector.tensor_tensor(pl[:], num[:], inv[:], op=Alu.mult)
            mask = epp.tile([128, 1], f32, name="mask")
            nc.vector.tensor_scalar(mask[:], cnt[:], 0.5, None, op0=Alu.is_ge)
            mask2 = epp.tile([128, 1], f32, name="mask2")
            nc.vector.tensor_tensor(mask2[:], mask[:], padmask[:], op=Alu.mult)
            plm = epp.tile([128, 1], f32, name="plm")
            nc.vector.scalar_tensor_tensor(plm[:], pl[:], 1.0 / float(D),
                                           mask2[:], op0=Alu.mult,
                                           op1=Alu.mult)
            if stage < 5:
                nc.sync.dma_start(out_d[:], plm[0:1, 0:1])
                return
            # partition-sum via 128x1 matmuls (f32)
            p_loss = psB.tile([1, 1], f32, tag="pl", name="p_loss")
            nc.tensor.matmul(p_loss[:], plm[:], ones_f[:])
            p_np = psB.tile([1, 1], f32, tag="pn", name="p_np")
            nc.tensor.matmul(p_np[:], mask2[:], ones_f[:])
            lm = epp.tile([1, 1], f32, name="lm")
            nc.vector.tensor_copy(lm[:], p_loss[:])
            npres = epp.tile([1, 1], f32, name="npres")
            nc.vector.tensor_copy(npres[:], p_np[:])
            # loss_core = lm - npres*a2sum/D  (the a2sum part of rowsum,
            # which is exactly -a2sum/D per present class)
            u1 = epp.tile([1, 1], f32, name="u1")
            nc.vector.tensor_tensor(u1[:], a2sum[:], npres[:], op=Alu.mult)
            lossc = epp.tile([1, 1], f32, name="lossc")
            nc.vector.scalar_tensor_tensor(lossc[:], u1[:], -1.0 / float(D),
                                           lm[:], op0=Alu.mult, op1=Alu.add)
            if stage < 6:
                nc.sync.dma_start(out_d[:], lossc[:])
                return
            fin = epp.tile([1, 16], f32, name="fin")
            nc.vector.memset(fin[:], 0.0)
            nc.vector.tensor_copy(fin[:, 0:1], lossc[:])
            f_in = dramp.tile([1, 16], f32, name="f_in")
            f_out = dramp.tile([1, 16], f32, name="f_out",
                               addr_space="Shared")
            nc.gpsimd.dma_start(f_in[:], fin[:])
            nc.gpsimd.collective_compute(
                "AllReduce", Alu.add, replica_groups=RG,
                ins=[f_in.opt()], outs=[f_out.opt()])
            nc.gpsimd.dma_start(out_d[:], f_out[:, 0:1])

    with tile.TileContext(nc, num_cores=N_CORES) as tc:
        _graph(tc)
    nc.compile()
    return nc


def _choose_boundaries(counts: np.ndarray) -> list[int]:
    """Split classes into N_CORES contiguous windows of <=PAD_SLOT classes,
    minimizing the max row count per window (binary search + greedy)."""
    prefix = np.concatenate([[0], np.cumsum(counts)]).astype(np.int64)
    total = int(prefix[-1])
    nclass = len(counts)

    def feasible(T):
        b = [0]
        c = 0
        for _ in range(N_CORES):
            hi = min(c + PAD_SLOT, nclass)
            c2 = int(np.searchsorted(prefix, prefix[c] + T, side="right") - 1)
            c2 = min(c2, hi)
            if c2 <= c:
                return None
            c = c2
            b.append(c)
            if c == nclass:
                break
        if c != nclass:
            return None
        while len(b) < N_CORES + 1:
            b.append(nclass)
        return b

    lo, hi = max(1, int(counts.max())), total
    while lo < hi:
        mid = (lo + hi) // 2
        if feasible(mid) is not None:
            hi = mid
        else:
            lo = mid + 1
    b = feasible(lo)
    assert b is not None
    return b


def _shard(x, anchors, y):
    x = np.asarray(x, dtype=np.float32)
    anchors = np.asarray(anchors, dtype=np.float32)
    y = np.asarray(y).astype(np.int64).ravel()

    counts = np.bincount(y, minlength=C)
    bounds = _choose_boundaries(counts)
    prefix = np.concatenate([[0], np.cumsum(counts)]).astype(np.int64)
    order = np.argsort(y, kind="stable")

    max_rows = max(int(prefix[bounds[j + 1]] - prefix[bounds[j]])
                   for j in range(N_CORES))
    rows_per_chunk = 128 * CHUNKS_PER_DMA
    nchunks = -(-max_rows // rows_per_chunk) * CHUNKS_PER_DMA
    nchunks = max(nchunks, CHUNKS_PER_DMA)
    R = nchunks * 128

    in_maps = []
    for j in range(N_CORES):
        c_lo, c_hi = bounds[j], bounds[j + 1]
        rows = order[prefix[c_lo]:prefix[c_hi]]
        nr = len(rows)
        xj = np.zeros((R, D), dtype=np.float32)
        xj[:nr] = x[rows]
        ylocal = np.full((R,), float(PAD_SLOT), dtype=np.float32)
        ylocal[:nr] = (y[rows] - c_lo).astype(np.float32)
        yj = np.ascontiguousarray(ylocal.reshape(nchunks, 128).T)
        alj = np.zeros((128, D), dtype=np.float32)
        alj[:c_hi - c_lo] = anchors[c_lo:c_hi]
        in_maps.append({"x": xj, "y": yj, "al": alj})
    return in_maps, nchunks


def _ensure_ntff_hook():
    """The agent image's `antenv` stub lacks `axon_hooks`, so trn_boot's
    NTFF registration silently degrades. Recreate the module and register
    the same ctypes-based hook so trace=True yields exec_time_ns."""
    import types

    if "antenv.axon_hooks" in sys.modules:
        return
    import antenv
    from trn_agent_boot.trn_boot import _ntff_profile_via_ctypes

    mod = types.ModuleType("antenv.axon_hooks")
    holder = [None]
    mod.set_axon_ntff_profile_hook = lambda h: holder.__setitem__(0, h)
    mod.get_axon_ntff_profile_hook = lambda: holder[0]
    sys.modules["antenv.axon_hooks"] = mod
    antenv.axon_hooks = mod
    mod.set_axon_ntff_profile_hook(
        _ntff_profile_via_ctypes("/opt/axon/libaxon_pjrt.so"))


def kernel(x, anchors, y, _trace=False, _trace_all=False):
    global LAST_EXEC_NS, LAST_RESULTS
    from concourse.bass_utils import run_bass_kernel_spmd

    if _trace:
        try:
            _ensure_ntff_hook()
        except Exception as e:  # tracing is best-effort
            print(f"ntff hook registration failed: {e}")

    in_maps, nchunks = _shard(x, anchors, y)
    nc = _build(nchunks)
    kw = {}
    if _trace:
        kw["trace"] = True
        if _trace_all:
            kw["trace_cores"] = list(range(N_CORES))
    res = run_bass_kernel_spmd(nc, in_maps, list(range(N_CORES)), **kw)
    LAST_EXEC_NS = res.exec_time_ns
    LAST_RESULTS = res
    return np.float32(res.results[0]["out"][0, 0])


# revision 18
# speedup vs baseline: 1.0942x; 1.0942x over previous
"""Distributed Trainium2 (Bass/Tile) kernel for nn_Anchor_Loss2.

Math: the reference computes
    dist[i,j] = (||x_i||^2 - 2 x_i.a_j + ||a_j||^2) / D
    S = segment_sum(dist, y); M = S / max(cnt,1)
    loss = sum_{l present} (2 M[l,l] - sum_j M[l,j])

Expanding the segment sum, only per-class aggregates are needed:
    cnt_l, sx2_l = sum_{i in l} ||x_i||^2, SX_l = sum_{i in l} x_i
    S[l,j]*D   = sx2_l - 2 SX_l.a_j + cnt_l*||a_j||^2
    rowsum_l*D = C*sx2_l - 2 SX_l.asum + cnt_l*a2sum
so the big GEMM x@anchors.T disappears; the dominant work is the
segment-sum of x (done as a one-hot matmul on TensorE).

Sharding: rows are assigned to cores BY CLASS (each core owns a
contiguous window of <=127 classes), so all rows of a class land on one
core and every per-class aggregate is fully local. Cross-core comm is
only (a) an all-reduce of the anchor column-sums (asum / a2sum come from
the full anchor set) and (b) an all-reduce of the final scalar loss.
Row->core assignment is a free choice of sharding since the loss is
permutation invariant in the rows.
"""

import functools
import os
import sys

import numpy as np

for _p in ("/opt/trn_rl_repo",):
    if _p not in sys.path:
        sys.path.insert(0, _p)

N_CORES = 8
C = 1000
D = 1024
PAD_SLOT = 127  # local class slot used for padding rows (always masked)
CHUNKS_PER_DMA = 4

LAST_EXEC_NS = None
LAST_RESULTS = None


@functools.lru_cache(maxsize=8)
def _build(nchunks: int, stage: int = 99):
    import concourse.bass as bass  # noqa: F401
    import concourse.mybir as mybir
    import concourse.tile as tile
    from concourse import bacc

    dt = mybir.dt
    f32 = dt.float32
    bf16 = dt.bfloat16
    i32 = dt.int32
    Alu = mybir.AluOpType
    AX = mybir.AxisListType

    R = nchunks * 128
    G = CHUNKS_PER_DMA
    assert nchunks % G == 0
    ngroups = nchunks // G

    nc = bacc.Bacc("TRN2", target_bir_lowering=False, debug=False,
                   num_devices=N_CORES)

    x_d = nc.dram_tensor("x", [R, D], f32, kind="ExternalInput")
    y_d = nc.dram_tensor("y", [128, nchunks], f32, kind="ExternalInput")
    al_d = nc.dram_tensor("al", [128, D], f32, kind="ExternalInput")
    out_d = nc.dram_tensor("out", [1, 1], f32, kind="ExternalOutput")

    RG = [list(range(N_CORES))]

    def _graph(tc):
        with (
            tc.tile_pool(name="const", bufs=1) as constp,
            tc.tile_pool(name="anch", bufs=1) as anchp,
            tc.tile_pool(name="xin", bufs=3) as xinp,
            tc.tile_pool(name="xbf", bufs=3) as xbfp,
            tc.tile_pool(name="sq", bufs=2) as sqp,
            tc.tile_pool(name="oh", bufs=4) as ohp,
            tc.tile_pool(name="sm", bufs=6) as smp,
            tc.tile_pool(name="ep", bufs=1) as epp,
            tc.tile_pool(name="psA", bufs=1, space="PSUM") as psA,
            tc.tile_pool(name="psB", bufs=1, space="PSUM") as psB,
            tc.tile_pool(name="dram", bufs=1, space="DRAM") as dramp,
        ):
            # ---------------- constants ----------------
            iota_i = constp.tile([128, 128], i32, name="iota_i")
            nc.gpsimd.iota(iota_i[:], pattern=[[1, 128]], base=0,
                           channel_multiplier=0)
            iota_f = constp.tile([128, 128], f32, name="iota_f")
            nc.vector.tensor_copy(iota_f[:], iota_i[:])
            ones_bf = constp.tile([128, 1], bf16, name="ones_bf")
            nc.vector.memset(ones_bf[:], 1.0)
            ones_f = constp.tile([128, 1], f32, name="ones_f")
            nc.vector.memset(ones_f[:], 1.0)
            pidx_i = constp.tile([128, 1], i32, name="pidx_i")
            nc.gpsimd.iota(pidx_i[:], pattern=[[1, 1]], base=0,
                           channel_multiplier=1)
            pidx_f = constp.tile([128, 1], f32, name="pidx_f")
            nc.vector.tensor_copy(pidx_f[:], pidx_i[:])
            padmask = constp.tile([128, 1], f32, name="padmask")
            nc.vector.tensor_scalar(padmask[:], pidx_f[:],
                                    float(PAD_SLOT) - 0.5, None,
                                    op0=Alu.is_le)
            y_sb = constp.tile([128, nchunks], f32, name="y_sb")
            nc.sync.dma_start(y_sb[:], y_d[:])

            # ---------- anchors: local slice + colsum all-reduce ----------
            A = anchp.tile([128, D], f32, name="A")
            nc.sync.dma_start(A[:], al_d[:])
            A_bf = anchp.tile([128, D], bf16, name="A_bf")
            nc.scalar.copy(A_bf[:], A[:])
            Asq_bf = anchp.tile([128, D], bf16, name="Asq_bf")
            nc.vector.tensor_tensor(Asq_bf[:], A_bf[:], A_bf[:], op=Alu.mult)
            cs_sb = anchp.tile([1, 2 * D], f32, name="cs_sb")
            for h in range(2):
                pcsa = psB.tile([1, 512], f32, tag="pcs", bufs=2,
                                name=f"pcsa{h}")
                nc.tensor.matmul(pcsa[:], ones_bf[:],
                                 A_bf[:, h * 512:(h + 1) * 512])
                nc.vector.tensor_copy(cs_sb[:, h * 512:(h + 1) * 512], pcsa[:])
            for h in range(2):
                pcsq = psB.tile([1, 512], f32, tag="pcs", bufs=2,
                                name=f"pcsq{h}")
                nc.tensor.matmul(pcsq[:], ones_bf[:],
                                 Asq_bf[:, h * 512:(h + 1) * 512])
                nc.vector.tensor_copy(cs_sb[:, D + h * 512:D + (h + 1) * 512],
                                      pcsq[:])
            if stage < 2:
                nc.sync.dma_start(out_d[:], cs_sb[:, 0:1])
                return
            cc_in = dramp.tile([1, 2 * D], f32, name="cc_in")
            cc_out = dramp.tile([1, 2 * D], f32, name="cc_out",
                                addr_space="Shared")
            nc.gpsimd.dma_start(cc_in[:], cs_sb[:])
            nc.gpsimd.collective_compute(
                "AllReduce", Alu.add, replica_groups=RG,
                ins=[cc_in.opt()], outs=[cc_out.opt()])
            ar_sb = anchp.tile([1, 2 * D], f32, name="ar_sb")
            nc.gpsimd.dma_start(ar_sb[:], cc_out[:])
            asum_bc = anchp.tile([128, D], f32, name="asum_bc")
            nc.gpsimd.partition_broadcast(asum_bc[:], ar_sb[:, 0:D])
            a2sum = anchp.tile([1, 1], f32, name="a2sum")
            nc.vector.tensor_reduce(a2sum[:], ar_sb[:, D:2 * D], axis=AX.X,
                                    op=Alu.add)

            if stage < 3:
                nc.sync.dma_start(out_d[:], a2sum[:])
                return
            # ---------------- main streaming accumulation ----------------
            p_sx0 = psA.tile([128, 512], f32, tag="sx0", name="p_sx0")
            p_sx1 = psA.tile([128, 512], f32, tag="sx1", name="p_sx1")
            p_sc = psA.tile([128, 2], f32, tag="sc", name="p_sc")

            x_r = x_d.ap().rearrange("(g t p) d -> g p t d", t=G, p=128)
            last_xb = [None]
            for g in range(ngroups):
                # SWDGE DMA converts f32 (HBM) -> bf16 (SBUF) in-flight,
                # so no on-chip cast op is needed at all.
                xb = xbfp.tile([128, G, D], bf16, name="xb")
                nc.gpsimd.dma_start(xb[:], x_r[g])
                last_xb[0] = xb
                for t in range(G):
                    k = g * G + t
                    st, sp = (k == 0), (k == nchunks - 1)
                    if stage >= 32:
                        oh_t = ohp.tile([128, 128], bf16, name="oh_t")
                        nc.vector.tensor_scalar(oh_t[:], iota_f[:],
                                                y_sb[:, k:k + 1], None,
                                                op0=Alu.is_equal)
                    if stage >= 33:
                        xsq = sqp.tile([128, D], bf16, name="xsq")
                        x2c = smp.tile([128, 1], f32, name="x2c")
                        if t % 2 == 0:
                            nc.scalar.activation(
                                xsq[:], xb[:, t, :],
                                mybir.ActivationFunctionType.Square,
                                accum_out=x2c[:])
                        else:
                            nc.vector.scalar_tensor_tensor(
                                xsq[:], xb[:, t, :], 1.0, xb[:, t, :],
                                op0=Alu.mult, op1=Alu.mult,
                                accum_out=x2c[:])
                        rhs2 = smp.tile([128, 2], bf16, name="rhs2")
                        # center: store (x2 - D) so the bf16 cast only sees
                        # the fluctuation; sx2 is rebuilt as D*cnt + sum
                        nc.vector.tensor_scalar_add(rhs2[:, 0:1], x2c[:],
                                                    -float(D))
                        nc.vector.memset(rhs2[:, 1:2], 1.0)
                    if stage >= 34:
                        nc.tensor.matmul(p_sx0[:], oh_t[:], xb[:, t, 0:512],
                                         start=st, stop=sp)
                        nc.tensor.matmul(p_sx1[:], oh_t[:], xb[:, t, 512:1024],
                                         start=st, stop=sp)
                        nc.tensor.matmul(p_sc[:], oh_t[:], rhs2[:],
                                         start=st, stop=sp)
            if stage < 34:
                res31 = epp.tile([1, 1], f32, name="res31")
                nc.vector.tensor_copy(res31[:], last_xb[0][0:1, 0, 0:1])
                nc.sync.dma_start(out_d[:], res31[:])
                return

            # ---------------- epilogue (per-class -> scalar) ----------------
            SX = epp.tile([128, D], f32, name="SX")
            nc.vector.tensor_copy(SX[:, 0:512], p_sx0[:])
            nc.vector.tensor_copy(SX[:, 512:1024], p_sx1[:])
            if stage < 4:
                nc.sync.dma_start(out_d[:], SX[0:1, 0:1])
                return
            x2cs = epp.tile([128, 1], f32, name="x2cs")
            nc.vector.tensor_copy(x2cs[:], p_sc[:, 0:1])
            cnt = epp.tile([128, 1], f32, name="cnt")
            nc.vector.tensor_copy(cnt[:], p_sc[:, 1:2])
            # sx2 = D*cnt + sum((x2 - D)) : undo the centering
            sx2 = epp.tile([128, 1], f32, name="sx2")
            nc.vector.scalar_tensor_tensor(sx2[:], cnt[:], float(D), x2cs[:],
                                           op0=Alu.mult, op1=Alu.add)
            scr = epp.tile([128, D], f32, name="scr")
            dotdiag = epp.tile([128, 1], f32, name="dotdiag")
            nc.vector.scalar_tensor_tensor(
                scr[:], SX[:], 1.0, A[:], op0=Alu.mult, op1=Alu.mult,
                accum_out=dotdiag[:])
            scr2 = epp.tile([128, D], f32, name="scr2")
            dotsum = epp.tile([128, 1], f32, name="dotsum")
            nc.vector.scalar_tensor_tensor(
                scr2[:], SX[:], 1.0, asum_bc[:], op0=Alu.mult, op1=Alu.mult,
                accum_out=dotsum[:])
            scr3 = epp.tile([128, D], f32, name="scr3")
            a2l = epp.tile([128, 1], f32, name="a2l")
            nc.vector.scalar_tensor_tensor(
                scr3[:], A[:], 1.0, A[:], op0=Alu.mult, op1=Alu.mult,
                accum_out=a2l[:])

            cntm = epp.tile([128, 1], f32, name="cntm")
            nc.vector.tensor_scalar_max(cntm[:], cnt[:], 1.0)
            inv = epp.tile([128, 1], f32, name="inv")
            nc.vector.reciprocal(inv[:], cntm[:])
            # t1 = sx2 - 2*dotdiag ; sll = t1 + cnt*a2l  (= S_ll * D)
            t1 = epp.tile([128, 1], f32, name="t1")
            nc.vector.scalar_tensor_tensor(t1[:], dotdiag[:], -2.0, sx2[:],
                                           op0=Alu.mult, op1=Alu.add)
            t2 = epp.tile([128, 1], f32, name="t2")
            nc.vector.tensor_tensor(t2[:], cnt[:], a2l[:], op=Alu.mult)
            sll = epp.tile([128, 1], f32, name="sll")
            nc.vector.tensor_tensor(sll[:], t1[:], t2[:], op=Alu.add)
            # rs = C*sx2 - 2*dotsum  (= rowsum*D minus the cnt*a2sum part)
            ds2 = epp.tile([128, 1], f32, name="ds2")
            nc.vector.tensor_scalar_mul(ds2[:], dotsum[:], 2.0)
            rs = epp.tile([128, 1], f32, name="rs")
            nc.vector.scalar_tensor_tensor(rs[:], sx2[:], float(C), ds2[:],
                                           op0=Alu.mult, op1=Alu.subtract)
            num = epp.tile([128, 1], f32, name="num")
            nc.vector.scalar_tensor_tensor(num[:], sll[:], 2.0, rs[:],
                                           op0=Alu.mult, op1=Alu.subtract)
            pl = epp.tile([128, 1], f32, name="pl")
            nc.vector.tensor_tensor(pl[:], num[:], inv[:], op=Alu.mult)
            mask = epp.tile([128, 1], f32, name="mask")
            nc.vector.tensor_scalar(mask[:], cnt[:], 0.5, None, op0=Alu.is_ge)
            mask2 = epp.tile([128, 1], f32, name="mask2")
            nc.vector.tensor_tensor(mask2[:], mask[:], padmask[:], op=Alu.mult)
            plm = epp.tile([128, 1], f32, name="plm")
            nc.vector.scalar_tensor_tensor(plm[:], pl[:], 1.0 / float(D),
                                           mask2[:], op0=Alu.mult,
                                           op1=Alu.mult)
            if stage < 5:
                nc.sync.dma_start(out_d[:], plm[0:1, 0:1])
                return
            # partition-sum via 128x1 matmuls (f32)
            p_loss = psB.tile([1, 1], f32, tag="pl", name="p_loss")
            nc.tensor.matmul(p_loss[:], plm[:], ones_f[:])
            p_np = psB.tile([1, 1], f32, tag="pn", name="p_np")
            nc.tensor.matmul(p_np[:], mask2[:], ones_f[:])
            lm = epp.tile([1, 1], f32, name="lm")
            nc.vector.tensor_copy(lm[:], p_loss[:])
            npres = epp.tile([1, 1], f32, name="npres")
            nc.vector.tensor_copy(npres[:], p_np[:])
            # loss_core = lm - npres*a2sum/D  (the a2sum part of rowsum,
            # which is exactly -a2sum/D per present class)
            u1 = epp.tile([1, 1], f32, name="u1")
            nc.vector.tensor_tensor(u1[:], a2sum[:], npres[:], op=Alu.mult)
            lossc = epp.tile([1, 1], f32, name="lossc")
            nc.vector.scalar_tensor_tensor(lossc[:], u1[:], -1.0 / float(D),
                                           lm[:], op0=Alu.mult, op1=Alu.add)
            if stage < 6:
                nc.sync.dma_start(out_d[:], lossc[:])
                return
            # final reduction via AllGather (lower floor than AllReduce)
            # + a tiny K=8 matmul to sum the gathered partials
            fin = epp.tile([1, 16], f32, name="fin")
            nc.vector.memset(fin[:], 0.0)
            nc.vector.tensor_copy(fin[:, 0:1], lossc[:])
            f_in = dramp.tile([1, 16], f32, name="f_in")
            f_out = dramp.tile([8, 16], f32, name="f_out",
                               addr_space="Shared")
            nc.gpsimd.dma_start(f_in[:], fin[:])
            nc.gpsimd.collective_compute(
                "AllGather", Alu.bypass, replica_groups=RG,
                ins=[f_in.opt()], outs=[f_out.opt()])
            ag_sb = epp.tile([8, 16], f32, name="ag_sb")
            nc.gpsimd.dma_start(ag_sb[:], f_out[:])
            p_fin = psB.tile([16, 1], f32, tag="pl", name="p_fin")
            nc.tensor.matmul(p_fin[:], ag_sb[:], ones_f[0:8, :])
            res_fin = epp.tile([1, 1], f32, name="res_fin")
            nc.vector.tensor_copy(res_fin[:], p_fin[0:1, :])
            nc.sync.dma_start(out_d[:], res_fin[:])

    with tile.TileContext(nc, num_cores=N_CORES) as tc:
        _graph(tc)
    nc.compile()
    return nc


def _choose_boundaries(counts: np.ndarray) -> list[int]:
    """Split classes into N_CORES contiguous windows of <=PAD_SLOT classes,
    minimizing the max row count per window (binary search + greedy)."""
    prefix = np.concatenate([[0], np.cumsum(counts)]).astype(np.int64)
    total = int(prefix[-1])
    nclass = len(counts)

    def feasible(T):
        b = [0]
        c = 0
        for _ in range(N_CORES):
            hi = min(c + PAD_SLOT, nclass)
            c2 = int(np.searchsorted(prefix, prefix[c] + T, side="right") - 1)
            c2 = min(c2, hi)
            if c2 <= c:
                return None
            c = c2
            b.append(c)
            if c == nclass:
                break
        if c != nclass:
            return None
        while len(b) < N_CORES + 1:
            b.append(nclass)
        return b

    lo, hi = max(1, int(counts.max())), total
    while lo < hi:
        mid = (lo + hi) // 2
        if feasible(mid) is not None:
            hi = mid
        else:
            lo = mid + 1
    b = feasible(lo)
    assert b is not None
    return b


def _shard(x, anchors, y):
    x = np.asarray(x, dtype=np.float32)
    anchors = np.asarray(anchors, dtype=np.float32)
    y = np.asarray(y).astype(np.int64).ravel()

    counts = np.bincount(y, minlength=C)
    bounds = _choose_boundaries(counts)
    prefix = np.concatenate([[0], np.cumsum(counts)]).astype(np.int64)
    order = np.argsort(y, kind="stable")

    max_rows = max(int(prefix[bounds[j + 1]] - prefix[bounds[j]])
                   for j in range(N_CORES))
    rows_per_chunk = 128 * CHUNKS_PER_DMA
    nchunks = -(-max_rows // rows_per_chunk) * CHUNKS_PER_DMA
    nchunks = max(nchunks, CHUNKS_PER_DMA)
    R = nchunks * 128

    in_maps = []
    for j in range(N_CORES):
        c_lo, c_hi = bounds[j], bounds[j + 1]
        rows = order[prefix[c_lo]:prefix[c_hi]]
        nr = len(rows)
        xj = np.zeros((R, D), dtype=np.float32)
        xj[:nr] = x[rows]
        ylocal = np.full((R,), float(PAD_SLOT), dtype=np.float32)
        ylocal[:nr] = (y[rows] - c_lo).astype(np.float32)
        yj = np.ascontiguousarray(ylocal.reshape(nchunks, 128).T)
        alj = np.zeros((128, D), dtype=np.float32)
        alj[:c_hi - c_lo] = anchors[c_lo:c_hi]
        in_maps.append({"x": xj, "y": yj, "al": alj})
    return in_maps, nchunks


def _ensure_ntff_hook():
    """The agent image's `antenv` stub lacks `axon_hooks`, so trn_boot's
    NTFF registration silently degrades. Recreate the module and register
    the same ctypes-based hook so trace=True yields exec_time_ns."""
    import types

    if "antenv.axon_hooks" in sys.modules:
        return
    import antenv
    from trn_agent_boot.trn_boot import _ntff_profile_via_ctypes

    mod = types.ModuleType("antenv.axon_hooks")
    holder = [None]
    mod.set_axon_ntff_profile_hook = lambda h: holder.__setitem__(0, h)
    mod.get_axon_ntff_profile_hook = lambda: holder[0]
    sys.modules["antenv.axon_hooks"] = mod
    antenv.axon_hooks = mod
    mod.set_axon_ntff_profile_hook(
        _ntff_profile_via_ctypes("/opt/axon/libaxon_pjrt.so"))


def kernel(x, anchors, y, _trace=False, _trace_all=False):
    global LAST_EXEC_NS, LAST_RESULTS
    from concourse.bass_utils import run_bass_kernel_spmd

    if _trace:
        try:
            _ensure_ntff_hook()
        except Exception as e:  # tracing is best-effort
            print(f"ntff hook registration failed: {e}")

    in_maps, nchunks = _shard(x, anchors, y)
    nc = _build(nchunks)
    kw = {}
    if _trace:
        kw["trace"] = True
        if _trace_all:
            kw["trace_cores"] = list(range(N_CORES))
    res = run_bass_kernel_spmd(nc, in_maps, list(range(N_CORES)), **kw)
    LAST_EXEC_NS = res.exec_time_ns
    LAST_RESULTS = res
    return np.float32(res.results[0]["out"][0, 0])


# revision 21
# speedup vs baseline: 1.1596x; 1.0597x over previous
"""Distributed Trainium2 (Bass/Tile) kernel for nn_Anchor_Loss2.

Math: the reference computes
    dist[i,j] = (||x_i||^2 - 2 x_i.a_j + ||a_j||^2) / D
    S = segment_sum(dist, y); M = S / max(cnt,1)
    loss = sum_{l present} (2 M[l,l] - sum_j M[l,j])

Expanding the segment sum, only per-class aggregates are needed:
    cnt_l, sx2_l = sum_{i in l} ||x_i||^2, SX_l = sum_{i in l} x_i
    S[l,j]*D   = sx2_l - 2 SX_l.a_j + cnt_l*||a_j||^2
    rowsum_l*D = C*sx2_l - 2 SX_l.asum + cnt_l*a2sum
so the big GEMM x@anchors.T disappears; the dominant work is the
segment-sum of x (done as a one-hot matmul on TensorE).

Sharding: rows are assigned to cores BY CLASS (each core owns a
contiguous window of <=127 classes), so all rows of a class land on one
core and every per-class aggregate is fully local. Cross-core comm is
only (a) an all-reduce of the anchor column-sums (asum / a2sum come from
the full anchor set) and (b) an all-reduce of the final scalar loss.
Row->core assignment is a free choice of sharding since the loss is
permutation invariant in the rows.
"""

import functools
import os
import sys

import numpy as np

for _p in ("/opt/trn_rl_repo",):
    if _p not in sys.path:
        sys.path.insert(0, _p)

N_CORES = 8
C = 1000
D = 1024
PAD_SLOT = 127  # local class slot used for padding rows (always masked)
CHUNKS_PER_DMA = 4

LAST_EXEC_NS = None
LAST_RESULTS = None


@functools.lru_cache(maxsize=8)
def _build(nchunks: int, stage: int = 99):
    import concourse.bass as bass  # noqa: F401
    import concourse.mybir as mybir
    import concourse.tile as tile
    from concourse import bacc

    dt = mybir.dt
    f32 = dt.float32
    bf16 = dt.bfloat16
    i32 = dt.int32
    Alu = mybir.AluOpType
    AX = mybir.AxisListType

    R = nchunks * 128
    G = CHUNKS_PER_DMA
    assert nchunks % G == 0
    ngroups = nchunks // G

    nc = bacc.Bacc("TRN2", target_bir_lowering=False, debug=False,
                   num_devices=N_CORES)

    x_d = nc.dram_tensor("x", [R, D], f32, kind="ExternalInput")
    y_d = nc.dram_tensor("y", [128, nchunks], f32, kind="ExternalInput")
    al_d = nc.dram_tensor("al", [128, D], f32, kind="ExternalInput")
    out_d = nc.dram_tensor("out", [1, 1], f32, kind="ExternalOutput")

    RG = [list(range(N_CORES))]

    def _graph(tc):
        with (
            tc.tile_pool(name="const", bufs=1) as constp,
            tc.tile_pool(name="anch", bufs=1) as anchp,
            tc.tile_pool(name="xin", bufs=3) as xinp,
            tc.tile_pool(name="xbf", bufs=3) as xbfp,
            tc.tile_pool(name="sq", bufs=2) as sqp,
            tc.tile_pool(name="oh", bufs=4) as ohp,
            tc.tile_pool(name="sm", bufs=6) as smp,
            tc.tile_pool(name="ep", bufs=1) as epp,
            tc.tile_pool(name="psA", bufs=1, space="PSUM") as psA,
            tc.tile_pool(name="psB", bufs=1, space="PSUM") as psB,
            tc.tile_pool(name="dram", bufs=1, space="DRAM") as dramp,
        ):
            # ---------------- constants ----------------
            iota_i = constp.tile([128, 128], i32, name="iota_i")
            nc.gpsimd.iota(iota_i[:], pattern=[[1, 128]], base=0,
                           channel_multiplier=0)
            iota_f = constp.tile([128, 128], f32, name="iota_f")
            nc.vector.tensor_copy(iota_f[:], iota_i[:])
            ones_bf = constp.tile([128, 1], bf16, name="ones_bf")
            nc.vector.memset(ones_bf[:], 1.0)
            ones_row_bf = constp.tile([1, 128], bf16, name="ones_row_bf")
            nc.vector.memset(ones_row_bf[:], 1.0)
            ones_f = constp.tile([128, 1], f32, name="ones_f")
            nc.vector.memset(ones_f[:], 1.0)
            pidx_i = constp.tile([128, 1], i32, name="pidx_i")
            nc.gpsimd.iota(pidx_i[:], pattern=[[1, 1]], base=0,
                           channel_multiplier=1)
            pidx_f = constp.tile([128, 1], f32, name="pidx_f")
            nc.vector.tensor_copy(pidx_f[:], pidx_i[:])
            padmask = constp.tile([128, 1], f32, name="padmask")
            nc.vector.tensor_scalar(padmask[:], pidx_f[:],
                                    float(PAD_SLOT) - 0.5, None,
                                    op0=Alu.is_le)
            y_sb = constp.tile([128, nchunks], f32, name="y_sb")
            nc.sync.dma_start(y_sb[:], y_d[:])

            # ---------- anchors: local slice + colsum all-reduce ----------
            A = anchp.tile([128, D], f32, name="A")
            nc.sync.dma_start(A[:], al_d[:])
            A_bf = anchp.tile([128, D], bf16, name="A_bf")
            nc.scalar.copy(A_bf[:], A[:])
            Asq_bf = anchp.tile([128, D], bf16, name="Asq_bf")
            nc.vector.tensor_tensor(Asq_bf[:], A_bf[:], A_bf[:], op=Alu.mult)
            cs_sb = anchp.tile([1, 2 * D], f32, name="cs_sb")
            for h in range(2):
                pcsa = psB.tile([1, 512], f32, tag="pcs", bufs=2,
                                name=f"pcsa{h}")
                nc.tensor.matmul(pcsa[:], ones_bf[:],
                                 A_bf[:, h * 512:(h + 1) * 512])
                nc.vector.tensor_copy(cs_sb[:, h * 512:(h + 1) * 512], pcsa[:])
            for h in range(2):
                pcsq = psB.tile([1, 512], f32, tag="pcs", bufs=2,
                                name=f"pcsq{h}")
                nc.tensor.matmul(pcsq[:], ones_bf[:],
                                 Asq_bf[:, h * 512:(h + 1) * 512])
                nc.vector.tensor_copy(cs_sb[:, D + h * 512:D + (h + 1) * 512],
                                      pcsq[:])
            if stage < 2:
                nc.sync.dma_start(out_d[:], cs_sb[:, 0:1])
                return
            # Only the non-blocking trigger goes on gpsimd here; every
            # consumer of the collective result is emitted in the epilogue
            # so no engine queue stalls behind the all-reduce while the
            # x-stream (SWDGE, also on gpsimd) runs.
            cc_in = dramp.tile([1, 2 * D], f32, name="cc_in")
            cc_out = dramp.tile([1, 2 * D], f32, name="cc_out",
                                addr_space="Shared")
            nc.gpsimd.dma_start(cc_in[:], cs_sb[:])
            nc.gpsimd.collective_compute(
                "AllReduce", Alu.add, replica_groups=RG,
                ins=[cc_in.opt()], outs=[cc_out.opt()])

            def anchor_epilogue():
                ar_sb = anchp.tile([1, 2 * D], f32, name="ar_sb")
                nc.sync.dma_start(ar_sb[:], cc_out[:])
                asum_bf = anchp.tile([1, D], bf16, name="asum_bf")
                nc.vector.tensor_copy(asum_bf[:], ar_sb[:, 0:D])
                asum_bc = anchp.tile([128, D], f32, name="asum_bc")
                for h in range(2):
                    pbc = psB.tile([128, 512], f32, tag="pcs", bufs=2,
                                   name=f"pbc{h}")
                    nc.tensor.matmul(pbc[:], ones_row_bf[:],
                                     asum_bf[:, h * 512:(h + 1) * 512])
                    nc.vector.tensor_copy(asum_bc[:, h * 512:(h + 1) * 512],
                                          pbc[:])
                a2sum = anchp.tile([1, 1], f32, name="a2sum")
                nc.vector.tensor_reduce(a2sum[:], ar_sb[:, D:2 * D],
                                        axis=AX.X, op=Alu.add)
                return asum_bc, a2sum

            if stage < 3:
                asum_bc, a2sum = anchor_epilogue()
                nc.sync.dma_start(out_d[:], a2sum[:])
                return
            # ---------------- main streaming accumulation ----------------
            p_sx0 = psA.tile([128, 512], f32, tag="sx0", name="p_sx0")
            p_sx1 = psA.tile([128, 512], f32, tag="sx1", name="p_sx1")
            p_sc = psA.tile([128, 2], f32, tag="sc", name="p_sc")

            x_r = x_d.ap().rearrange("(g t p) d -> g p t d", t=G, p=128)
            last_xb = [None]
            for g in range(ngroups):
                # SWDGE DMA converts f32 (HBM) -> bf16 (SBUF) in-flight,
                # so no on-chip cast op is needed at all.
                xb = xbfp.tile([128, G, D], bf16, name="xb")
                nc.gpsimd.dma_start(xb[:], x_r[g])
                last_xb[0] = xb
                for t in range(G):
                    k = g * G + t
                    st, sp = (k == 0), (k == nchunks - 1)
                    if stage >= 32:
                        oh_t = ohp.tile([128, 128], bf16, name="oh_t")
                        nc.vector.tensor_scalar(oh_t[:], iota_f[:],
                                                y_sb[:, k:k + 1], None,
                                                op0=Alu.is_equal)
                    if stage >= 33:
                        xsq = sqp.tile([128, D], bf16, name="xsq")
                        x2c = smp.tile([128, 1], f32, name="x2c")
                        if t % 2 == 0:
                            nc.scalar.activation(
                                xsq[:], xb[:, t, :],
                                mybir.ActivationFunctionType.Square,
                                accum_out=x2c[:])
                        else:
                            nc.vector.scalar_tensor_tensor(
                                xsq[:], xb[:, t, :], 1.0, xb[:, t, :],
                                op0=Alu.mult, op1=Alu.mult,
                                accum_out=x2c[:])
                        rhs2 = smp.tile([128, 2], bf16, name="rhs2")
                        # center: store (x2 - D) so the bf16 cast only sees
                        # the fluctuation; sx2 is rebuilt as D*cnt + sum
                        nc.vector.tensor_scalar_add(rhs2[:, 0:1], x2c[:],
                                                    -float(D))
                        nc.vector.memset(rhs2[:, 1:2], 1.0)
                    if stage >= 34:
                        nc.tensor.matmul(p_sx0[:], oh_t[:], xb[:, t, 0:512],
                                         start=st, stop=sp)
                        nc.tensor.matmul(p_sx1[:], oh_t[:], xb[:, t, 512:1024],
                                         start=st, stop=sp)
                        nc.tensor.matmul(p_sc[:], oh_t[:], rhs2[:],
                                         start=st, stop=sp)
            if stage < 34:
                res31 = epp.tile([1, 1], f32, name="res31")
                nc.vector.tensor_copy(res31[:], last_xb[0][0:1, 0, 0:1])
                nc.sync.dma_start(out_d[:], res31[:])
                return

            # ---------------- epilogue (per-class -> scalar) ----------------
            asum_bc, a2sum = anchor_epilogue()
            SX = epp.tile([128, D], f32, name="SX")
            nc.vector.tensor_copy(SX[:, 0:512], p_sx0[:])
            nc.vector.tensor_copy(SX[:, 512:1024], p_sx1[:])
            if stage < 4:
                nc.sync.dma_start(out_d[:], SX[0:1, 0:1])
                return
            x2cs = epp.tile([128, 1], f32, name="x2cs")
            nc.vector.tensor_copy(x2cs[:], p_sc[:, 0:1])
            cnt = epp.tile([128, 1], f32, name="cnt")
            nc.vector.tensor_copy(cnt[:], p_sc[:, 1:2])
            # sx2 = D*cnt + sum((x2 - D)) : undo the centering
            sx2 = epp.tile([128, 1], f32, name="sx2")
            nc.vector.scalar_tensor_tensor(sx2[:], cnt[:], float(D), x2cs[:],
                                           op0=Alu.mult, op1=Alu.add)
            scr = epp.tile([128, D], f32, name="scr")
            dotdiag = epp.tile([128, 1], f32, name="dotdiag")
            nc.vector.scalar_tensor_tensor(
                scr[:], SX[:], 1.0, A[:], op0=Alu.mult, op1=Alu.mult,
                accum_out=dotdiag[:])
            scr2 = epp.tile([128, D], f32, name="scr2")
            dotsum = epp.tile([128, 1], f32, name="dotsum")
            nc.vector.scalar_tensor_tensor(
                scr2[:], SX[:], 1.0, asum_bc[:], op0=Alu.mult, op1=Alu.mult,
                accum_out=dotsum[:])
            scr3 = epp.tile([128, D], f32, name="scr3")
            a2l = epp.tile([128, 1], f32, name="a2l")
            nc.vector.scalar_tensor_tensor(
                scr3[:], A[:], 1.0, A[:], op0=Alu.mult, op1=Alu.mult,
                accum_out=a2l[:])

            cntm = epp.tile([128, 1], f32, name="cntm")
            nc.vector.tensor_scalar_max(cntm[:], cnt[:], 1.0)
            inv = epp.tile([128, 1], f32, name="inv")
            nc.vector.reciprocal(inv[:], cntm[:])
            # t1 = sx2 - 2*dotdiag ; sll = t1 + cnt*a2l  (= S_ll * D)
            t1 = epp.tile([128, 1], f32, name="t1")
            nc.vector.scalar_tensor_tensor(t1[:], dotdiag[:], -2.0, sx2[:],
                                           op0=Alu.mult, op1=Alu.add)
            t2 = epp.tile([128, 1], f32, name="t2")
            nc.vector.tensor_tensor(t2[:], cnt[:], a2l[:], op=Alu.mult)
            sll = epp.tile([128, 1], f32, name="sll")
            nc.vector.tensor_tensor(sll[:], t1[:], t2[:], op=Alu.add)
            # rs = C*sx2 - 2*dotsum  (= rowsum*D minus the cnt*a2sum part)
            ds2 = epp.tile([128, 1], f32, name="ds2")
            nc.vector.tensor_scalar_mul(ds2[:], dotsum[:], 2.0)
            rs = epp.tile([128, 1], f32, name="rs")
            nc.vector.scalar_tensor_tensor(rs[:], sx2[:], float(C), ds2[:],
                                           op0=Alu.mult, op1=Alu.subtract)
            num = epp.tile([128, 1], f32, name="num")
            nc.vector.scalar_tensor_tensor(num[:], sll[:], 2.0, rs[:],
                                           op0=Alu.mult, op1=Alu.subtract)
            pl = epp.tile([128, 1], f32, name="pl")
            nc.vector.tensor_tensor(pl[:], num[:], inv[:], op=Alu.mult)
            mask = epp.tile([128, 1], f32, name="mask")
            nc.vector.tensor_scalar(mask[:], cnt[:], 0.5, None, op0=Alu.is_ge)
            mask2 = epp.tile([128, 1], f32, name="mask2")
            nc.vector.tensor_tensor(mask2[:], mask[:], padmask[:], op=Alu.mult)
            plm = epp.tile([128, 1], f32, name="plm")
            nc.vector.scalar_tensor_tensor(plm[:], pl[:], 1.0 / float(D),
                                           mask2[:], op0=Alu.mult,
                                           op1=Alu.mult)
            if stage < 5:
                nc.sync.dma_start(out_d[:], plm[0:1, 0:1])
                return
            # partition-sum via 128x1 matmuls (f32)
            p_loss = psB.tile([1, 1], f32, tag="pl", name="p_loss")
            nc.tensor.matmul(p_loss[:], plm[:], ones_f[:])
            p_np = psB.tile([1, 1], f32, tag="pn", name="p_np")
            nc.tensor.matmul(p_np[:], mask2[:], ones_f[:])
            lm = epp.tile([1, 1], f32, name="lm")
            nc.vector.tensor_copy(lm[:], p_loss[:])
            npres = epp.tile([1, 1], f32, name="npres")
            nc.vector.tensor_copy(npres[:], p_np[:])
            # loss_core = lm - npres*a2sum/D  (the a2sum part of rowsum,
            # which is exactly -a2sum/D per present class)
            u1 = epp.tile([1, 1], f32, name="u1")
            nc.vector.tensor_tensor(u1[:], a2sum[:], npres[:], op=Alu.mult)
            lossc = epp.tile([1, 1], f32, name="lossc")
            nc.vector.scalar_tensor_tensor(lossc[:], u1[:], -1.0 / float(D),
                                           lm[:], op0=Alu.mult, op1=Alu.add)
            if stage < 6:
                nc.sync.dma_start(out_d[:], lossc[:])
                return
            # final reduction via AllGather (lower floor than AllReduce)
            # + a tiny K=8 matmul to sum the gathered partials
            fin = epp.tile([1, 16], f32, name="fin")
            nc.vector.memset(fin[:], 0.0)
            nc.vector.tensor_copy(fin[:, 0:1], lossc[:])
            f_in = dramp.tile([1, 16], f32, name="f_in")
            f_out = dramp.tile([8, 16], f32, name="f_out",
                               addr_space="Shared")
            nc.gpsimd.dma_start(f_in[:], fin[:])
            nc.gpsimd.collective_compute(
                "AllGather", Alu.bypass, replica_groups=RG,
                ins=[f_in.opt()], outs=[f_out.opt()])
            ag_sb = epp.tile([8, 16], f32, name="ag_sb")
            nc.gpsimd.dma_start(ag_sb[:], f_out[:])
            p_fin = psB.tile([16, 1], f32, tag="pl", name="p_fin")
            nc.tensor.matmul(p_fin[:], ag_sb[:], ones_f[0:8, :])
            res_fin = epp.tile([1, 1], f32, name="res_fin")
            nc.vector.tensor_copy(res_fin[:], p_fin[0:1, :])
            nc.sync.dma_start(out_d[:], res_fin[:])

    with tile.TileContext(nc, num_cores=N_CORES) as tc:
        _graph(tc)
    nc.compile()
    return nc


def _choose_boundaries(counts: np.ndarray) -> list[int]:
    """Split classes into N_CORES contiguous windows of <=PAD_SLOT classes,
    minimizing the max row count per window (binary search + greedy)."""
    prefix = np.concatenate([[0], np.cumsum(counts)]).astype(np.int64)
    total = int(prefix[-1])
    nclass = len(counts)

    def feasible(T):
        b = [0]
        c = 0
        for _ in range(N_CORES):
            hi = min(c + PAD_SLOT, nclass)
            c2 = int(np.searchsorted(prefix, prefix[c] + T, side="right") - 1)
            c2 = min(c2, hi)
            if c2 <= c:
                return None
            c = c2
            b.append(c)
            if c == nclass:
                break
        if c != nclass:
            return None
        while len(b) < N_CORES + 1:
            b.append(nclass)
        return b

    lo, hi = max(1, int(counts.max())), total
    while lo < hi:
        mid = (lo + hi) // 2
        if feasible(mid) is not None:
            hi = mid
        else:
            lo = mid + 1
    b = feasible(lo)
    assert b is not None
    return b


def _shard(x, anchors, y):
    x = np.asarray(x, dtype=np.float32)
    anchors = np.asarray(anchors, dtype=np.float32)
    y = np.asarray(y).astype(np.int64).ravel()

    counts = np.bincount(y, minlength=C)
    bounds = _choose_boundaries(counts)
    prefix = np.concatenate([[0], np.cumsum(counts)]).astype(np.int64)
    order = np.argsort(y, kind="stable")

    max_rows = max(int(prefix[bounds[j + 1]] - prefix[bounds[j]])
                   for j in range(N_CORES))
    rows_per_chunk = 128 * CHUNKS_PER_DMA
    nchunks = -(-max_rows // rows_per_chunk) * CHUNKS_PER_DMA
    nchunks = max(nchunks, CHUNKS_PER_DMA)
    R = nchunks * 128

    in_maps = []
    for j in range(N_CORES):
        c_lo, c_hi = bounds[j], bounds[j + 1]
        rows = order[prefix[c_lo]:prefix[c_hi]]
        nr = len(rows)
        xj = np.zeros((R, D), dtype=np.float32)
        xj[:nr] = x[rows]
        ylocal = np.full((R,), float(PAD_SLOT), dtype=np.float32)
        ylocal[:nr] = (y[rows] - c_lo).astype(np.float32)
        yj = np.ascontiguousarray(ylocal.reshape(nchunks, 128).T)
        alj = np.zeros((128, D), dtype=np.float32)
        alj[:c_hi - c_lo] = anchors[c_lo:c_hi]
        in_maps.append({"x": xj, "y": yj, "al": alj})
    return in_maps, nchunks


def _ensure_ntff_hook():
    """The agent image's `antenv` stub lacks `axon_hooks`, so trn_boot's
    NTFF registration silently degrades. Recreate the module and register
    the same ctypes-based hook so trace=True yields exec_time_ns."""
    import types

    if "antenv.axon_hooks" in sys.modules:
        return
    import antenv
    from trn_agent_boot.trn_boot import _ntff_profile_via_ctypes

    mod = types.ModuleType("antenv.axon_hooks")
    holder = [None]
    mod.set_axon_ntff_profile_hook = lambda h: holder.__setitem__(0, h)
    mod.get_axon_ntff_profile_hook = lambda: holder[0]
    sys.modules["antenv.axon_hooks"] = mod
    antenv.axon_hooks = mod
    mod.set_axon_ntff_profile_hook(
        _ntff_profile_via_ctypes("/opt/axon/libaxon_pjrt.so"))


def kernel(x, anchors, y, _trace=False, _trace_all=False):
    global LAST_EXEC_NS, LAST_RESULTS
    from concourse.bass_utils import run_bass_kernel_spmd

    if _trace:
        try:
            _ensure_ntff_hook()
        except Exception as e:  # tracing is best-effort
            print(f"ntff hook registration failed: {e}")

    in_maps, nchunks = _shard(x, anchors, y)
    nc = _build(nchunks)
    kw = {}
    if _trace:
        kw["trace"] = True
        if _trace_all:
            kw["trace_cores"] = list(range(N_CORES))
    res = run_bass_kernel_spmd(nc, in_maps, list(range(N_CORES)), **kw)
    LAST_EXEC_NS = res.exec_time_ns
    LAST_RESULTS = res
    return np.float32(res.results[0]["out"][0, 0])


# revision 26
# speedup vs baseline: 1.1749x; 1.0132x over previous
"""Distributed Trainium2 (Bass/Tile) kernel for nn_Anchor_Loss2.

Math: the reference computes
    dist[i,j] = (||x_i||^2 - 2 x_i.a_j + ||a_j||^2) / D
    S = segment_sum(dist, y); M = S / max(cnt,1)
    loss = sum_{l present} (2 M[l,l] - sum_j M[l,j])

Expanding the segment sum, only per-class aggregates are needed:
    cnt_l, sx2_l = sum_{i in l} ||x_i||^2, SX_l = sum_{i in l} x_i
    S[l,j]*D   = sx2_l - 2 SX_l.a_j + cnt_l*||a_j||^2
    rowsum_l*D = C*sx2_l - 2 SX_l.asum + cnt_l*a2sum
so the big GEMM x@anchors.T disappears; the dominant work is the
segment-sum of x (done as a one-hot matmul on TensorE).

Sharding: rows are assigned to cores BY CLASS (each core owns a
contiguous window of <=127 classes), so all rows of a class land on one
core and every per-class aggregate is fully local. Cross-core comm is
only (a) an all-reduce of the anchor column-sums (asum / a2sum come from
the full anchor set) and (b) an all-reduce of the final scalar loss.
Row->core assignment is a free choice of sharding since the loss is
permutation invariant in the rows.
"""

import functools
import os
import sys

import numpy as np

for _p in ("/opt/trn_rl_repo",):
    if _p not in sys.path:
        sys.path.insert(0, _p)

N_CORES = 8
C = 1000
D = 1024
PAD_SLOT = 127  # local class slot used for padding rows (always masked)
CHUNKS_PER_DMA = 4

LAST_EXEC_NS = None
LAST_RESULTS = None


@functools.lru_cache(maxsize=8)
def _build(nchunks: int, stage: int = 99):
    import concourse.bass as bass  # noqa: F401
    import concourse.mybir as mybir
    import concourse.tile as tile
    from concourse import bacc

    dt = mybir.dt
    f32 = dt.float32
    bf16 = dt.bfloat16
    i32 = dt.int32
    Alu = mybir.AluOpType
    AX = mybir.AxisListType

    R = nchunks * 128
    G = CHUNKS_PER_DMA
    assert nchunks % G == 0
    ngroups = nchunks // G

    nc = bacc.Bacc("TRN2", target_bir_lowering=False, debug=False,
                   num_devices=N_CORES)

    x_d = nc.dram_tensor("x", [R, D], f32, kind="ExternalInput")
    y_d = nc.dram_tensor("y", [128, nchunks], f32, kind="ExternalInput")
    al_d = nc.dram_tensor("al", [128, D], f32, kind="ExternalInput")
    af_d = nc.dram_tensor("af", [1024, D], f32, kind="ExternalInput")
    out_d = nc.dram_tensor("out", [1, 1], f32, kind="ExternalOutput")

    RG = [list(range(N_CORES))]

    def _graph(tc):
        with (
            tc.tile_pool(name="const", bufs=1) as constp,
            tc.tile_pool(name="anch", bufs=1) as anchp,
            tc.tile_pool(name="xin", bufs=3) as xinp,
            tc.tile_pool(name="xbf", bufs=3) as xbfp,
            tc.tile_pool(name="sq", bufs=2) as sqp,
            tc.tile_pool(name="oh", bufs=4) as ohp,
            tc.tile_pool(name="sm", bufs=6) as smp,
            tc.tile_pool(name="ep", bufs=1) as epp,
            tc.tile_pool(name="psA", bufs=1, space="PSUM") as psA,
            tc.tile_pool(name="psB", bufs=1, space="PSUM") as psB,
            tc.tile_pool(name="dram", bufs=1, space="DRAM") as dramp,
        ):
            # ---------------- constants ----------------
            iota_i = constp.tile([128, 128], i32, name="iota_i")
            nc.gpsimd.iota(iota_i[:], pattern=[[1, 128]], base=0,
                           channel_multiplier=0)
            iota_f = constp.tile([128, 128], f32, name="iota_f")
            nc.vector.tensor_copy(iota_f[:], iota_i[:])
            ones_bf = constp.tile([128, 1], bf16, name="ones_bf")
            nc.vector.memset(ones_bf[:], 1.0)
            ones_row_bf = constp.tile([1, 128], bf16, name="ones_row_bf")
            nc.vector.memset(ones_row_bf[:], 1.0)
            ones_f = constp.tile([128, 1], f32, name="ones_f")
            nc.vector.memset(ones_f[:], 1.0)
            pidx_i = constp.tile([128, 1], i32, name="pidx_i")
            nc.gpsimd.iota(pidx_i[:], pattern=[[1, 1]], base=0,
                           channel_multiplier=1)
            pidx_f = constp.tile([128, 1], f32, name="pidx_f")
            nc.vector.tensor_copy(pidx_f[:], pidx_i[:])
            padmask = constp.tile([128, 1], f32, name="padmask")
            nc.vector.tensor_scalar(padmask[:], pidx_f[:],
                                    float(PAD_SLOT) - 0.5, None,
                                    op0=Alu.is_le)
            y_sb = constp.tile([128, nchunks], f32, name="y_sb")
            nc.sync.dma_start(y_sb[:], y_d[:])

            # ---------- anchors: every core holds the full set, so asum
            # and a2sum are computed locally (no mid-stream collective,
            # which would starve the SDMA engines while it runs) ----------
            A = anchp.tile([128, D], f32, name="A")
            nc.sync.dma_start(A[:], al_d[:])
            # full anchors, cast to bf16 in-flight, as 8 row-blocks
            af_bf = anchp.tile([128, 8, D], bf16, name="af_bf")
            af_r = af_d.ap().rearrange("(b p) d -> p b d", p=128)
            nc.gpsimd.dma_start(af_bf[:], af_r)
            # colsum_A[d] = sum_c A[c,d] : accumulate 8 ones-matmuls
            p_csa0 = psB.tile([1, 512], f32, tag="pcs", bufs=2, name="p_csa0")
            p_csa1 = psB.tile([1, 512], f32, tag="pcs", bufs=2, name="p_csa1")
            for b in range(8):
                nc.tensor.matmul(p_csa0[:], ones_bf[:], af_bf[:, b, 0:512],
                                 start=(b == 0), stop=(b == 7))
                nc.tensor.matmul(p_csa1[:], ones_bf[:], af_bf[:, b, 512:1024],
                                 start=(b == 0), stop=(b == 7))
            asum_bf = anchp.tile([1, D], bf16, name="asum_bf")
            nc.vector.tensor_copy(asum_bf[:, 0:512], p_csa0[:])
            nc.vector.tensor_copy(asum_bf[:, 512:1024], p_csa1[:])
            # a2sum = sum over all anchors of a^2 via 8 ACT square+accum
            a2acc = anchp.tile([128, 8], f32, name="a2acc")
            afsq = anchp.tile([128, D], bf16, name="afsq")
            for b in range(8):
                nc.scalar.activation(afsq[:], af_bf[:, b, :],
                                     mybir.ActivationFunctionType.Square,
                                     accum_out=a2acc[:, b:b + 1])
            a2row = anchp.tile([128, 1], f32, name="a2row")
            nc.vector.tensor_reduce(a2row[:], a2acc[:], axis=AX.X, op=Alu.add)
            p_a2 = psB.tile([1, 1], f32, tag="pn", name="p_a2")
            nc.tensor.matmul(p_a2[:], a2row[:], ones_f[:])
            a2sum = anchp.tile([1, 1], f32, name="a2sum")
            nc.vector.tensor_copy(a2sum[:], p_a2[:])
            # broadcast asum to all 128 partitions via K=1 matmul
            asum_bc = anchp.tile([128, D], f32, name="asum_bc")
            for h in range(2):
                pbc = psB.tile([128, 512], f32, tag="pcs", bufs=2,
                               name=f"pbc{h}")
                nc.tensor.matmul(pbc[:], ones_row_bf[:],
                                 asum_bf[:, h * 512:(h + 1) * 512])
                nc.vector.tensor_copy(asum_bc[:, h * 512:(h + 1) * 512],
                                      pbc[:])
            if stage < 3:
                nc.sync.dma_start(out_d[:], a2sum[:])
                return
            # ---------------- main streaming accumulation ----------------
            p_sx0 = psA.tile([128, 512], f32, tag="sx0", name="p_sx0")
            p_sx1 = psA.tile([128, 512], f32, tag="sx1", name="p_sx1")
            p_sc = psA.tile([128, 2], f32, tag="sc", name="p_sc")

            x_r = x_d.ap().rearrange("(g t p) d -> g p t d", t=G, p=128)
            last_xb = [None]
            for g in range(ngroups):
                # SWDGE DMA converts f32 (HBM) -> bf16 (SBUF) in-flight,
                # so no on-chip cast op is needed at all.
                xb = xbfp.tile([128, G, D], bf16, name="xb")
                nc.gpsimd.dma_start(xb[:], x_r[g])
                last_xb[0] = xb
                for t in range(G):
                    k = g * G + t
                    st, sp = (k == 0), (k == nchunks - 1)
                    if stage >= 32:
                        oh_t = ohp.tile([128, 128], bf16, name="oh_t")
                        nc.vector.tensor_scalar(oh_t[:], iota_f[:],
                                                y_sb[:, k:k + 1], None,
                                                op0=Alu.is_equal)
                    if stage >= 33:
                        xsq = sqp.tile([128, D], bf16, name="xsq")
                        x2c = smp.tile([128, 1], f32, name="x2c")
                        if t % 2 == 0:
                            nc.scalar.activation(
                                xsq[:], xb[:, t, :],
                                mybir.ActivationFunctionType.Square,
                                accum_out=x2c[:])
                        else:
                            nc.vector.scalar_tensor_tensor(
                                xsq[:], xb[:, t, :], 1.0, xb[:, t, :],
                                op0=Alu.mult, op1=Alu.mult,
                                accum_out=x2c[:])
                        rhs2 = smp.tile([128, 2], bf16, name="rhs2")
                        # center: store (x2 - D) so the bf16 cast only sees
                        # the fluctuation; sx2 is rebuilt as D*cnt + sum
                        nc.vector.tensor_scalar_add(rhs2[:, 0:1], x2c[:],
                                                    -float(D))
                        nc.vector.memset(rhs2[:, 1:2], 1.0)
                    if stage >= 34:
                        nc.tensor.matmul(p_sx0[:], oh_t[:], xb[:, t, 0:512],
                                         start=st, stop=sp)
                        nc.tensor.matmul(p_sx1[:], oh_t[:], xb[:, t, 512:1024],
                                         start=st, stop=sp)
                        nc.tensor.matmul(p_sc[:], oh_t[:], rhs2[:],
                                         start=st, stop=sp)
            if stage < 34:
                res31 = epp.tile([1, 1], f32, name="res31")
                nc.vector.tensor_copy(res31[:], last_xb[0][0:1, 0, 0:1])
                nc.sync.dma_start(out_d[:], res31[:])
                return

            # ---------------- epilogue (per-class -> scalar) ----------------
            SX = epp.tile([128, D], f32, name="SX")
            nc.vector.tensor_copy(SX[:, 0:512], p_sx0[:])
            nc.vector.tensor_copy(SX[:, 512:1024], p_sx1[:])
            if stage < 4:
                nc.sync.dma_start(out_d[:], SX[0:1, 0:1])
                return
            x2cs = epp.tile([128, 1], f32, name="x2cs")
            nc.vector.tensor_copy(x2cs[:], p_sc[:, 0:1])
            cnt = epp.tile([128, 1], f32, name="cnt")
            nc.vector.tensor_copy(cnt[:], p_sc[:, 1:2])
            # sx2 = D*cnt + sum((x2 - D)) : undo the centering
            sx2 = epp.tile([128, 1], f32, name="sx2")
            nc.vector.scalar_tensor_tensor(sx2[:], cnt[:], float(D), x2cs[:],
                                           op0=Alu.mult, op1=Alu.add)
            scr = epp.tile([128, D], f32, name="scr")
            dotdiag = epp.tile([128, 1], f32, name="dotdiag")
            nc.vector.scalar_tensor_tensor(
                scr[:], SX[:], 1.0, A[:], op0=Alu.mult, op1=Alu.mult,
                accum_out=dotdiag[:])
            scr2 = epp.tile([128, D], f32, name="scr2")
            dotsum = epp.tile([128, 1], f32, name="dotsum")
            nc.vector.scalar_tensor_tensor(
                scr2[:], SX[:], 1.0, asum_bc[:], op0=Alu.mult, op1=Alu.mult,
                accum_out=dotsum[:])
            scr3 = epp.tile([128, D], f32, name="scr3")
            a2l = epp.tile([128, 1], f32, name="a2l")
            nc.vector.scalar_tensor_tensor(
                scr3[:], A[:], 1.0, A[:], op0=Alu.mult, op1=Alu.mult,
                accum_out=a2l[:])

            cntm = epp.tile([128, 1], f32, name="cntm")
            nc.vector.tensor_scalar_max(cntm[:], cnt[:], 1.0)
            inv = epp.tile([128, 1], f32, name="inv")
            nc.vector.reciprocal(inv[:], cntm[:])
            # t1 = sx2 - 2*dotdiag ; sll = t1 + cnt*a2l  (= S_ll * D)
            t1 = epp.tile([128, 1], f32, name="t1")
            nc.vector.scalar_tensor_tensor(t1[:], dotdiag[:], -2.0, sx2[:],
                                           op0=Alu.mult, op1=Alu.add)
            t2 = epp.tile([128, 1], f32, name="t2")
            nc.vector.tensor_tensor(t2[:], cnt[:], a2l[:], op=Alu.mult)
            sll = epp.tile([128, 1], f32, name="sll")
            nc.vector.tensor_tensor(sll[:], t1[:], t2[:], op=Alu.add)
            # rs = C*sx2 - 2*dotsum  (= rowsum*D minus the cnt*a2sum part)
            ds2 = epp.tile([128, 1], f32, name="ds2")
            nc.vector.tensor_scalar_mul(ds2[:], dotsum[:], 2.0)
            rs = epp.tile([128, 1], f32, name="rs")
            nc.vector.scalar_tensor_tensor(rs[:], sx2[:], float(C), ds2[:],
                                           op0=Alu.mult, op1=Alu.subtract)
            num = epp.tile([128, 1], f32, name="num")
            nc.vector.scalar_tensor_tensor(num[:], sll[:], 2.0, rs[:],
                                           op0=Alu.mult, op1=Alu.subtract)
            pl = epp.tile([128, 1], f32, name="pl")
            nc.vector.tensor_tensor(pl[:], num[:], inv[:], op=Alu.mult)
            mask = epp.tile([128, 1], f32, name="mask")
            nc.vector.tensor_scalar(mask[:], cnt[:], 0.5, None, op0=Alu.is_ge)
            mask2 = epp.tile([128, 1], f32, name="mask2")
            nc.vector.tensor_tensor(mask2[:], mask[:], padmask[:], op=Alu.mult)
            plm = epp.tile([128, 1], f32, name="plm")
            nc.vector.scalar_tensor_tensor(plm[:], pl[:], 1.0 / float(D),
                                           mask2[:], op0=Alu.mult,
                                           op1=Alu.mult)
            if stage < 5:
                nc.sync.dma_start(out_d[:], plm[0:1, 0:1])
                return
            # partition-sum via 128x1 matmuls (f32)
            p_loss = psB.tile([1, 1], f32, tag="pl", name="p_loss")
            nc.tensor.matmul(p_loss[:], plm[:], ones_f[:])
            p_np = psB.tile([1, 1], f32, tag="pn", name="p_np")
            nc.tensor.matmul(p_np[:], mask2[:], ones_f[:])
            lm = epp.tile([1, 1], f32, name="lm")
            nc.vector.tensor_copy(lm[:], p_loss[:])
            npres = epp.tile([1, 1], f32, name="npres")
            nc.vector.tensor_copy(npres[:], p_np[:])
            # loss_core = lm - npres*a2sum/D  (the a2sum part of rowsum,
            # which is exactly -a2sum/D per present class)
            u1 = epp.tile([1, 1], f32, name="u1")
            nc.vector.tensor_tensor(u1[:], a2sum[:], npres[:], op=Alu.mult)
            lossc = epp.tile([1, 1], f32, name="lossc")
            nc.vector.scalar_tensor_tensor(lossc[:], u1[:], -1.0 / float(D),
                                           lm[:], op0=Alu.mult, op1=Alu.add)
            if stage < 6:
                nc.sync.dma_start(out_d[:], lossc[:])
                return
            # final reduction via AllGather (lower floor than AllReduce)
            # + a tiny K=8 matmul to sum the gathered partials
            fin = epp.tile([1, 16], f32, name="fin")
            nc.vector.memset(fin[:], 0.0)
            nc.vector.tensor_copy(fin[:, 0:1], lossc[:])
            f_in = dramp.tile([1, 16], f32, name="f_in")
            f_out = dramp.tile([8, 16], f32, name="f_out",
                               addr_space="Shared")
            nc.gpsimd.dma_start(f_in[:], fin[:])
            nc.gpsimd.collective_compute(
                "AllGather", Alu.bypass, replica_groups=RG,
                ins=[f_in.opt()], outs=[f_out.opt()])
            ag_sb = epp.tile([8, 16], f32, name="ag_sb")
            nc.gpsimd.dma_start(ag_sb[:], f_out[:])
            p_fin = psB.tile([16, 1], f32, tag="pl", name="p_fin")
            nc.tensor.matmul(p_fin[:], ag_sb[:], ones_f[0:8, :])
            res_fin = epp.tile([1, 1], f32, name="res_fin")
            nc.vector.tensor_copy(res_fin[:], p_fin[0:1, :])
            nc.sync.dma_start(out_d[:], res_fin[:])

    with tile.TileContext(nc, num_cores=N_CORES) as tc:
        _graph(tc)
    nc.compile()
    return nc


def _choose_boundaries(counts: np.ndarray) -> list[int]:
    """Split classes into N_CORES contiguous windows of <=PAD_SLOT classes,
    minimizing the max row count per window (binary search + greedy)."""
    prefix = np.concatenate([[0], np.cumsum(counts)]).astype(np.int64)
    total = int(prefix[-1])
    nclass = len(counts)

    def feasible(T):
        b = [0]
        c = 0
        for _ in range(N_CORES):
            hi = min(c + PAD_SLOT, nclass)
            c2 = int(np.searchsorted(prefix, prefix[c] + T, side="right") - 1)
            c2 = min(c2, hi)
            if c2 <= c:
                return None
            c = c2
            b.append(c)
            if c == nclass:
                break
        if c != nclass:
            return None
        while len(b) < N_CORES + 1:
            b.append(nclass)
        return b

    lo, hi = max(1, int(counts.max())), total
    while lo < hi:
        mid = (lo + hi) // 2
        if feasible(mid) is not None:
            hi = mid
        else:
            lo = mid + 1
    b = feasible(lo)
    assert b is not None
    return b


def _shard(x, anchors, y):
    x = np.asarray(x, dtype=np.float32)
    anchors = np.asarray(anchors, dtype=np.float32)
    y = np.asarray(y).astype(np.int64).ravel()

    counts = np.bincount(y, minlength=C)
    bounds = _choose_boundaries(counts)
    prefix = np.concatenate([[0], np.cumsum(counts)]).astype(np.int64)
    order = np.argsort(y, kind="stable")

    max_rows = max(int(prefix[bounds[j + 1]] - prefix[bounds[j]])
                   for j in range(N_CORES))
    rows_per_chunk = 128 * CHUNKS_PER_DMA
    nchunks = -(-max_rows // rows_per_chunk) * CHUNKS_PER_DMA
    nchunks = max(nchunks, CHUNKS_PER_DMA)
    R = nchunks * 128

    afull = np.zeros((1024, D), dtype=np.float32)
    afull[:C] = anchors
    in_maps = []
    for j in range(N_CORES):
        c_lo, c_hi = bounds[j], bounds[j + 1]
        rows = order[prefix[c_lo]:prefix[c_hi]]
        nr = len(rows)
        xj = np.zeros((R, D), dtype=np.float32)
        xj[:nr] = x[rows]
        ylocal = np.full((R,), float(PAD_SLOT), dtype=np.float32)
        ylocal[:nr] = (y[rows] - c_lo).astype(np.float32)
        yj = np.ascontiguousarray(ylocal.reshape(nchunks, 128).T)
        alj = np.zeros((128, D), dtype=np.float32)
        alj[:c_hi - c_lo] = anchors[c_lo:c_hi]
        in_maps.append({"x": xj, "y": yj, "al": alj, "af": afull})
    return in_maps, nchunks


def _ensure_ntff_hook():
    """The agent image's `antenv` stub lacks `axon_hooks`, so trn_boot's
    NTFF registration silently degrades. Recreate the module and register
    the same ctypes-based hook so trace=True yields exec_time_ns."""
    import types

    if "antenv.axon_hooks" in sys.modules:
        return
    import antenv
    from trn_agent_boot.trn_boot import _ntff_profile_via_ctypes

    mod = types.ModuleType("antenv.axon_hooks")
    holder = [None]
    mod.set_axon_ntff_profile_hook = lambda h: holder.__setitem__(0, h)
    mod.get_axon_ntff_profile_hook = lambda: holder[0]
    sys.modules["antenv.axon_hooks"] = mod
    antenv.axon_hooks = mod
    mod.set_axon_ntff_profile_hook(
        _ntff_profile_via_ctypes("/opt/axon/libaxon_pjrt.so"))


def kernel(x, anchors, y, _trace=False, _trace_all=False):
    global LAST_EXEC_NS, LAST_RESULTS
    from concourse.bass_utils import run_bass_kernel_spmd

    if _trace:
        try:
            _ensure_ntff_hook()
        except Exception as e:  # tracing is best-effort
            print(f"ntff hook registration failed: {e}")

    in_maps, nchunks = _shard(x, anchors, y)
    nc = _build(nchunks)
    kw = {}
    if _trace:
        kw["trace"] = True
        if _trace_all:
            kw["trace_cores"] = list(range(N_CORES))
    res = run_bass_kernel_spmd(nc, in_maps, list(range(N_CORES)), **kw)
    LAST_EXEC_NS = res.exec_time_ns
    LAST_RESULTS = res
    return np.float32(res.results[0]["out"][0, 0])
